# revision 1
# baseline (speedup 1.0000x reference)
"""Trainium2 Bass kernel for nn_Attention_40570261078258.

Computes, for x:(8,128,64,64), Wq/Wk/Wv:(128,128), bq/bk/bv:(128,):
    xf = x.reshape(N, C, L);  L = 4096
    q/k/v = W @ xf + b                  -> (N, L, C) logical
    scores = q @ k^T / sqrt(C)          -> (N, L, L)
    attn = softmax(scores, axis=0)      # over the BATCH axis (torch legacy dim=0)
    out = attn @ v                      -> (N, L, C)
    return x + out.reshape(N, C, H, W)  # reinterpreting (L,C) memory as (C,H,W)

Sharding: the softmax couples all batch elements at each (l, m) pair, so
batch-parallel would need a 64MB denominator all-reduce. Instead we shard the
query dim L across the 8 cores: each core handles l in [d*512, (d+1)*512) for
ALL batch elements, making the softmax entirely local (no collectives).
Each core redundantly computes k/v for all of L (cheap vs. attention).

SPMD: all cores run the identical graph; the per-core slice is selected by the
host passing a per-core q-input slice (xq). The device returns the attention
output in (c,l)-major tiles; the host reinterleaves and adds the residual.
"""

import math

import numpy as np

import concourse.bacc as bacc
import concourse.bass as bass
import concourse.mybir as mybir
import concourse.tile as tile
from concourse.bass_utils import run_bass_kernel_spmd

N, C, H, W = 8, 128, 64, 64
L = H * W            # 4096 pixels
NCORES = 8
LSH = L // NCORES    # 512 query positions per core
NLH = 2              # l-halves per core
LHW = LSH // NLH     # 256 l per half
NMT = L // 128       # 32 key/value tiles of 128

FP = mybir.dt.float32
FR = mybir.dt.float32r
BF = mybir.dt.bfloat16
AF = mybir.ActivationFunctionType

SKEW = 4           # m-tiles of lookahead between scores and softmax/AV
MUL_POOL_GROUPS = 4   # of the 8 normalize-mul batch groups, how many on gpsimd
REPEAT = 1         # benchmarking: emit the attention phase this many times
E_BUFS = SKEW + 3  # E-tile slots (4KB/partition each)
A_BUFS = SKEW + 1  # attn-tile slots
ZT_BUFS = 3        # z-chain temp slots
POOL_MUL_SPLIT = 4  # batch-groups per gpsimd normalize-mul op
MUL_FLAT = False   # normalize-mul as 8 plain 2D ops instead of 3D broadcast
BENCH_INTERNAL = False  # benchmarking: x + out in internal DRAM (no transfer)

# Set by test harness to capture a profile.
TRACE = False
LAST_RESULTS = None


def build():
    nc = bacc.Bacc(
        "TRN2",
        target_bir_lowering=False,
        debug=False,
        enable_asserts=True,
        num_devices=NCORES,
    )

    # x and the transposed weights are declared float32r (same bits as f32)
    # so the projection matmuls run at full PE rate without a bf16 pre-cast.
    if not BENCH_INTERNAL:
        xk = nc.dram_tensor("xk", [N, C, L], FR, kind="ExternalInput").ap()
        xq = nc.dram_tensor("xq", [N, C, LSH], FR, kind="ExternalInput").ap()
    else:
        xk = nc.dram_tensor("xk_i", [N, C, L], FR, kind="Internal").ap()
        xq = nc.dram_tensor("xq_i", [N, C, LSH], FR, kind="Internal").ap()
    # Weights arrive pre-transposed from the host: w*t[c, o] = W[o, c].
    wq = nc.dram_tensor("wqt", [C, C], FR, kind="ExternalInput").ap()
    wk = nc.dram_tensor("wkt", [C, C], FR, kind="ExternalInput").ap()
    wv = nc.dram_tensor("wvt", [C, C], FR, kind="ExternalInput").ap()
    bq = nc.dram_tensor("bq", [C, 1], FP, kind="ExternalInput").ap()
    bk = nc.dram_tensor("bk", [C, 1], FP, kind="ExternalInput").ap()
    bv = nc.dram_tensor("bv", [1, C], FP, kind="ExternalInput").ap()
    # Attention output in (c, l)-major layout; the host does the cheap
    # (l,c) reinterleave + residual add (pure glue, 0.4% of the FLOPs).
    if not BENCH_INTERNAL:
        out = nc.dram_tensor(
            "out", [N, NLH, C, LHW], FP, kind="ExternalOutput"
        ).ap()
    else:
        out = nc.dram_tensor(
            "out_i", [N, NLH, C, LHW], FP, kind="Internal"
        ).ap()
        tok = nc.dram_tensor("tok", [1, 4], FP, kind="ExternalOutput").ap()

    with tile.TileContext(nc) as tc:
        if BENCH_INTERNAL:
            # Zero the internal x so exp() sees sane values; one-time cost,
            # constant across variants (cancels in the repeat slope).
            with tc.tile_pool(name="zinit", bufs=1) as zp:
                zt = zp.tile([128, 2048], FP, tag="z0")
                nc.vector.memset(zt[:], 0.0)
                xkf = xk.rearrange("n c l -> (n c) l").rearrange(
                    "(b p) l -> b p l", p=128
                )
                for b in range(xkf.shape[0]):
                    for c0 in range(0, xkf.shape[2], 2048):
                        nc.sync.dma_start(
                            xkf[b, :, c0 : c0 + 2048].bitcast(FP), zt[:]
                        )
                xqf = xq.rearrange("n c l -> (n c) l").rearrange(
                    "(b p) l -> b p l", p=128
                )
                for b in range(xqf.shape[0]):
                    nc.sync.dma_start(xqf[b].bitcast(FP), zt[:, : xqf.shape[2]])
                nc.sync.dma_start(tok, zt[0:1, 0:4])
        _emit(nc, tc, xk, xq, wq, wk, wv, bq, bk, bv, out)

    nc.compile()
    return nc


def _emit(nc, tc, xk, xq, wq, wk, wv, bq, bk, bv, out):
    from contextlib import ExitStack

    with ExitStack() as ctx:
        cpool = ctx.enter_context(tc.tile_pool(name="const", bufs=1))
        resid = ctx.enter_context(tc.tile_pool(name="resident", bufs=1))

        # --- constants -----------------------------------------------------
        bq_t = cpool.tile([C, 1], FP, tag="bq")
        nc.sync.dma_start(bq_t[:], bq)
        bk_t = cpool.tile([C, 1], FP, tag="bk")
        nc.sync.dma_start(bk_t[:], bk)
        bv_f = cpool.tile([1, C], FP, tag="bvf")
        nc.sync.dma_start(bv_f[:], bv)
        ones_row = cpool.tile([1, C], FP, tag="ones")
        nc.vector.memset(ones_row[:], 1.0)
        # bv replicated across partitions (rank-1 ones @ bv matmul)
        bv_rep = cpool.tile([128, C], FP, tag="bvrep")

        # Pre-transposed weights (WT[c, o] = W[o, c], f32r bits straight
        # from the host); projections run as float32r matmuls (full PE rate
        # at free dim >= 256, ~fp32 precision, no bf16 pre-cast of x).
        wT = {}
        with tc.tile_pool(name="wpsum", bufs=1, space="PSUM") as wpsum_pool:
            for name, wap in (("q", wq), ("k", wk), ("v", wv)):
                wt = cpool.tile([C, C], FR, tag=f"w{name}T")
                nc.sync.dma_start(wt[:], wap)
                wT[name] = wt
            pb = wpsum_pool.tile([128, C], FP, tag="wps")
            nc.tensor.matmul(pb[:], ones_row[:], bv_f[:], start=True, stop=True)
            nc.vector.tensor_copy(bv_rep[:], pb[:])
        # WvT padded to 256 columns of zeros so the float32r vT matmuls hit
        # the >=256 free-dim full-rate path (junk half never read).
        wvpad = cpool.tile([C, 2 * C], FR, tag="wvpad")
        zpad = cpool.tile([C, 2 * C], FP, tag="zpad")
        nc.vector.memset(zpad[:], 0.0)
        nc.vector.tensor_copy(wvpad[:], zpad[:])
        nc.vector.tensor_copy(wvpad[:, 0:C], wT["v"][:])

        # --- resident activations -----------------------------------------
        # q_sb[n]: (c, l) for this core's l-slice;  k_sb[n]: (c, m) full L;
        # vT_sb[n]: (m % 128, 32*128) i.e. 32 chunks of (m,c), all bf16.
        q_sb = [
            resid.tile([C, LSH], BF, tag=f"q{n}", name=f"q_sb{n}") for n in range(N)
        ]
        k_sb = [
            resid.tile([C, L], BF, tag=f"k{n}", name=f"k_sb{n}") for n in range(N)
        ]
        vT_sb = [
            resid.tile([128, NMT * C], BF, tag=f"v{n}", name=f"vT_sb{n}")
            for n in range(N)
        ]

        # --- phase 1: projections (float32r matmuls straight from f32r x) ---
        wqT_r = wT["q"][:]
        wkT_r = wT["k"][:]
        wvpad_r = wvpad[:]
        with (
            tc.tile_pool(name="xin", bufs=3) as xin_pool,
            tc.tile_pool(name="pj", bufs=2, space="PSUM") as pj_psum,
            tc.tile_pool(name="pv", bufs=2, space="PSUM") as pv_psum,
        ):
            for n in range(N):
                # q from the per-core slice
                xt = xin_pool.tile([C, LSH], FR, tag="x")
                nc.sync.dma_start(xt[:], xq[n])
                pq = pj_psum.tile([128, 1024], FP, tag="pj")
                nc.tensor.matmul(
                    pq[:, 0:512], wqT_r, xt[:], start=True, stop=True
                )
                nc.scalar.activation(
                    q_sb[n][:], pq[:, 0:512], AF.Identity, bias=bq_t[:]
                )

                for qch in range(L // 2048):
                    xt = xin_pool.tile([C, 2048], FR, tag="x")
                    nc.sync.dma_start(
                        xt[:], xk[n, :, qch * 2048 : (qch + 1) * 2048]
                    )

                    for hb in range(2):
                        bch = 2 * qch + hb
                        xr_ = xt[:, hb * 1024 : (hb + 1) * 1024]

                        # Both 512-wide k matmuls land in one 2-bank PSUM tile
                        # so the bias-adding eviction is one 1024-wide op.
                        pk = pj_psum.tile([128, 1024], FP, tag="pj")
                        for half in range(2):
                            nc.tensor.matmul(
                                pk[:, half * 512 : (half + 1) * 512],
                                wkT_r,
                                xr_[:, half * 512 : (half + 1) * 512],
                                start=True,
                                stop=True,
                            )
                        nc.scalar.activation(
                            k_sb[n][:, bch * 1024 : (bch + 1) * 1024],
                            pk[:],
                            AF.Identity,
                            bias=bk_t[:],
                        )

                        for half in range(2):
                            ch = 2 * bch + half
                            # vT chunks: out[m,c] = sum_c' x[c',m] WvT[c',c] + bv[c]
                            # Each 128-m sub-tile occupies a 256-wide PSUM slice
                            # (f32r full-rate needs >=256 free; upper half junk).
                            # One group per 2KB bank: `start` only on the first
                            # matmul into each bank.
                            pv = pv_psum.tile([128, 1024], FP, tag="pv")
                            for sub in range(4):
                                sl = slice(sub * 256, sub * 256 + 256)
                                nc.tensor.matmul(
                                    pv[:, sl],
                                    xr_[:, half * 512 + sub * 128 :
                                        half * 512 + (sub + 1) * 128],
                                    wvpad_r,
                                    start=(sub % 2 == 0),
                                    stop=(sub % 2 == 1),
                                )
                            # DVE (idle during phase 1) takes the vT eviction;
                            # strided 3D read picks the real 128 of each 256, and
                            # the bv bias rides along via the broadcast add.
                            nc.vector.scalar_tensor_tensor(
                                vT_sb[n][:, ch * 512 : (ch + 1) * 512].rearrange(
                                    "p (s c) -> p s c", s=4
                                ),
                                pv[:].rearrange("p (s c2) -> p s c2", s=4)[
                                    :, :, 0:128
                                ],
                                1.0,
                                bv_rep[:].unsqueeze(1).broadcast_to((128, 4, C)),
                                mybir.AluOpType.mult,
                                mybir.AluOpType.add,
                            )

        # --- phase 2: attention with softmax over batch --------------------
        inv_sqrt_c = 1.0 / math.sqrt(C)
        with (
            tc.tile_pool(name="scp", bufs=2, space="PSUM") as sc_psum,
            tc.tile_pool(name="avp", bufs=2, space="PSUM") as av_psum,
            tc.tile_pool(name="soft", bufs=1) as soft_pool,
            tc.tile_pool(name="ost", bufs=1) as ost_pool,
        ):
            def emit_epilogue(avp_prev, lh_prev):
                # Evict PSUM accumulators (freeing the av slots for the next
                # l-half) straight to DRAM in (c, l)-major layout.
                for n in range(N):
                    j, i = n // 4, n % 4
                    ob = ost_pool.tile([128, LHW], FP, tag="ob", bufs=4)
                    nc.vector.tensor_copy(
                        ob[:], avp_prev[j][:, i * LHW : (i + 1) * LHW]
                    )
                    nc.sync.dma_start(out[n, lh_prev], ob[:])

            pend = {}   # (lh, mt) -> E tile (128, 8n x 256l)
            avps = {}   # lh -> accumulator tiles

            def emit_scores(lh, mt):
                l0 = lh * LHW
                e = soft_pool.tile([128, 2048], BF, tag="E", bufs=E_BUFS)
                for j in range(2):
                    ps = sc_psum.tile([128, 1024], FP, tag="sc")
                    for i in range(4):
                        n = 4 * j + i
                        nc.tensor.matmul(
                            ps[:, i * LHW : (i + 1) * LHW],
                            k_sb[n][:, mt * 128 : (mt + 1) * 128],
                            q_sb[n][:, l0 : l0 + LHW],
                            start=True,
                            stop=True,
                        )
                    nc.scalar.activation(
                        e[:, j * 1024 : (j + 1) * 1024],
                        ps[:],
                        AF.Exp,
                        scale=inv_sqrt_c,
                    )
                pend[(lh, mt)] = e

            def emit_soft_av(lh, mt):
                if mt == 0:
                    # Two (c, 4n x 256l) accumulators, 2 PSUM banks each;
                    # group start/stop is per 2KB bank.
                    avps[lh] = [
                        av_psum.tile(
                            [128, 1024], FP, tag="av", name=f"avp{lh}_{j}"
                        )
                        for j in range(2)
                    ]
                avp = avps[lh]
                e = pend.pop((lh, mt))
                if True:
                    s1 = soft_pool.tile([128, 1024], BF, tag="zt1", bufs=ZT_BUFS)
                    nc.vector.tensor_add(s1[:], e[:, 0:1024], e[:, 1024:2048])
                    s2 = soft_pool.tile([128, 512], BF, tag="zt2", bufs=ZT_BUFS)
                    nc.vector.tensor_add(s2[:], s1[:, 0:512], s1[:, 512:1024])
                    zr = soft_pool.tile([128, LHW], BF, tag="zr", bufs=4)
                    nc.vector.tensor_add(zr[:], s2[:, 0:LHW], s2[:, LHW : 2 * LHW])
                    r = soft_pool.tile([128, LHW], BF, tag="r", bufs=4)
                    with nc.allow_low_precision(
                        "softmax denom is a sum of 8 O(1..500) exps; bf16 ok"
                    ):
                        nc.vector.reciprocal(r[:], zr[:])
                    # attn[n] = E[n] * (1/Z) via stride-0 broadcast of r along
                    # the batch-group dim; n0-3 on DVE, n4-7 on the otherwise
                    # idle gpsimd (each half feeds its own AV matmuls, so the
                    # slower engine's latency pipelines away).
                    a = soft_pool.tile([128, 2048], BF, tag="A", bufs=A_BUFS)
                    gd = 8 - MUL_POOL_GROUPS
                    if gd:
                        nc.vector.tensor_mul(
                            a[:, : gd * LHW].rearrange("p (g l) -> p g l", g=gd),
                            e[:, : gd * LHW].rearrange("p (g l) -> p g l", g=gd),
                            r[:].unsqueeze(1).broadcast_to((128, gd, LHW)),
                        )
                    for g0 in range(gd, 8, POOL_MUL_SPLIT):
                        g1 = min(g0 + POOL_MUL_SPLIT, 8)
                        gp = g1 - g0
                        nc.gpsimd.tensor_mul(
                            a[:, g0 * LHW : g1 * LHW].rearrange(
                                "p (g l) -> p g l", g=gp
                            ),
                            e[:, g0 * LHW : g1 * LHW].rearrange(
                                "p (g l) -> p g l", g=gp
                            ),
                            r[:].unsqueeze(1).broadcast_to((128, gp, LHW)),
                        )
                    for j in range(2):
                        for i in range(4):
                            n = 4 * j + i
                            sl = slice(i * LHW, (i + 1) * LHW)
                            nc.tensor.matmul(
                                avp[j][:, sl],
                                vT_sb[n][:, mt * C : (mt + 1) * C],
                                a[:, n * LHW : (n + 1) * LHW],
                                start=(mt == 0 and i % 2 == 0),
                                stop=(mt == NMT - 1 and i % 2 == 1),
                            )
                if mt == NMT - 1:
                    emit_epilogue(avps.pop(lh), lh)

            # One flat software-pipelined stream over all (lh, mt) jobs; the
            # scores stream runs SKEW jobs ahead of softmax/AV, including
            # across the l-half boundary, so no pipeline drain in between.
            jobs = [
                (lh, mt)
                for _ in range(REPEAT)
                for lh in range(NLH)
                for mt in range(NMT)
            ]
            for t, job in enumerate(jobs):
                emit_scores(*job)
                if t >= SKEW:
                    emit_soft_av(*jobs[t - SKEW])
            for job in jobs[len(jobs) - SKEW :]:
                emit_soft_av(*job)


_NC = None


def _get_nc():
    global _NC
    if _NC is None:
        _NC = build()
    return _NC


def kernel(x, Wq, bq, Wk, bk, Wv, bv):
    global LAST_RESULTS
    x = np.ascontiguousarray(np.asarray(x, dtype=np.float32))
    WqT = np.ascontiguousarray(np.asarray(Wq, dtype=np.float32).T)
    WkT = np.ascontiguousarray(np.asarray(Wk, dtype=np.float32).T)
    WvT = np.ascontiguousarray(np.asarray(Wv, dtype=np.float32).T)
    bq = np.asarray(bq, dtype=np.float32).reshape(C, 1)
    bk = np.asarray(bk, dtype=np.float32).reshape(C, 1)
    bv = np.asarray(bv, dtype=np.float32).reshape(1, C)

    xf = x.reshape(N, C, L)
    xflat = x.reshape(N, C * H * W)

    in_maps = []
    for d in range(NCORES):
        lo = d * LSH
        in_maps.append(
            {
                "xk": xf,
                "xq": np.ascontiguousarray(xf[:, :, lo : lo + LSH]),
                "wqt": WqT,
                "wkt": WkT,
                "wvt": WvT,
                "bq": bq,
                "bk": bk,
                "bv": bv,
            }
        )

    nc = _get_nc()
    res = run_bass_kernel_spmd(
        nc, in_maps, core_ids=list(range(NCORES)), trace=TRACE
    )
    LAST_RESULTS = res
    # Device returns attention output in (c, l)-major tiles; reinterleave to
    # the reference's flat (l, c) order and add the residual here.
    att = np.concatenate(
        [
            res.results[d]["out"].transpose(0, 1, 3, 2).reshape(N, LSH * C)
            for d in range(NCORES)
        ],
        axis=1,
    )
    return (xflat + att).reshape(N, C, H, W)



# revision 46
# speedup vs baseline: 1.0101x; 1.0101x over previous
"""Trainium2 Bass kernel for nn_Attention_40570261078258.

Computes, for x:(8,128,64,64), Wq/Wk/Wv:(128,128), bq/bk/bv:(128,):
    xf = x.reshape(N, C, L);  L = 4096
    q/k/v = W @ xf + b                  -> (N, L, C) logical
    scores = q @ k^T / sqrt(C)          -> (N, L, L)
    attn = softmax(scores, axis=0)      # over the BATCH axis (torch legacy dim=0)
    out = attn @ v                      -> (N, L, C)
    return x + out.reshape(N, C, H, W)  # reinterpreting (L,C) memory as (C,H,W)

Sharding: the softmax couples all batch elements at each (l, m) pair, so
batch-parallel would need a 64MB denominator all-reduce. Instead we shard the
query dim L across the 8 cores: each core handles l in [d*512, (d+1)*512) for
ALL batch elements, making the softmax entirely local (no collectives).
Each core redundantly computes k/v for all of L (cheap vs. attention).

Engine assignment (per the TimelineSim cost model):
  PE   : projections (f32r) + scores (bf16) + attn@v (bf16)     ~152us
  ACT  : the 16.7M-element exp, and nothing else                ~134us
  DVE  : softmax tree + reciprocal + normalize (stt 4x paths),
         q-hat eviction, AV eviction                            ~110us
  POOL : k-hat / v-hat PSUM evictions (bias-add casts)          ~104us
The softmax elementwise chain uses scalar_tensor_tensor ops (4x DVE rate for
all-SBUF bf16 operands) instead of tensor_tensor (2x ceiling).

SPMD: all cores run the identical graph; the per-core slice is selected by the
host passing a per-core q-input slice (xq). The device returns the attention
output in (c,l)-major tiles; the host reinterleaves and adds the residual.
"""

import math

import numpy as np

import concourse.bacc as bacc
import concourse.bass as bass
import concourse.mybir as mybir
import concourse.tile as tile
from concourse.bass_utils import run_bass_kernel_spmd

N, C, H, W = 8, 128, 64, 64
L = H * W            # 4096 pixels
NCORES = 8
LSH = L // NCORES    # 512 query positions per core
NLH = 4              # l-quarters per core
LHW = LSH // NLH     # 128 l per quarter
NMT = L // 128       # 32 key/value tiles of 128

FP = mybir.dt.float32
FR = mybir.dt.float32r
BF = mybir.dt.bfloat16
AF = mybir.ActivationFunctionType
ALU = mybir.AluOpType

SKEW = 6           # jobs of lookahead between scores and softmax/AV
E_BUFS = SKEW // 2 + 2  # E pair-tile slots (4KB/partition each)
A_BUFS = 3         # attn pair-tile slots
MUL_GD = 4         # normalize-mul batch groups on DVE (rest on gpsimd)

# Set by test harness to capture a profile.
TRACE = False
LAST_RESULTS = None


def build():
    nc = bacc.Bacc(
        "TRN2",
        target_bir_lowering=False,
        debug=False,
        enable_asserts=True,
        num_devices=NCORES,
    )

    # x and the transposed weights are declared float32r (same bits as f32)
    # so the projection matmuls run at full PE rate without a bf16 pre-cast.
    xk = nc.dram_tensor("xk", [N, C, L], FR, kind="ExternalInput").ap()
    xq = nc.dram_tensor("xq", [N, C, LSH], FR, kind="ExternalInput").ap()
    # Weights arrive pre-transposed from the host: w*t[c, o] = W[o, c].
    wq = nc.dram_tensor("wqt", [C, C], FR, kind="ExternalInput").ap()
    wk = nc.dram_tensor("wkt", [C, C], FR, kind="ExternalInput").ap()
    wv = nc.dram_tensor("wvt", [C, C], FR, kind="ExternalInput").ap()
    bq = nc.dram_tensor("bq", [C, 1], FP, kind="ExternalInput").ap()
    bk = nc.dram_tensor("bk", [C, 1], FP, kind="ExternalInput").ap()
    bv = nc.dram_tensor("bv", [1, C], FP, kind="ExternalInput").ap()
    # Attention output in (c, l)-major layout, bf16 (the host upcasts during
    # the (l,c) reinterleave + residual add; out values are O(20) so bf16
    # rounding adds ~2e-3 relative error, well inside the 2e-2 gate).
    out = nc.dram_tensor("out", [N, NLH, C, LHW], BF, kind="ExternalOutput").ap()

    with tile.TileContext(nc) as tc:
        _emit(nc, tc, xk, xq, wq, wk, wv, bq, bk, bv, out)

    nc.compile()
    return nc


def _emit(nc, tc, xk, xq, wq, wk, wv, bq, bk, bv, out):
    from contextlib import ExitStack

    with ExitStack() as ctx:
        cpool = ctx.enter_context(tc.tile_pool(name="const", bufs=1))
        resid = ctx.enter_context(tc.tile_pool(name="resident", bufs=1))

        # --- constants -----------------------------------------------------
        bq_t = cpool.tile([C, 1], FP, tag="bq")
        nc.sync.dma_start(bq_t[:], bq)
        bk_t = cpool.tile([C, 1], FP, tag="bk")
        nc.sync.dma_start(bk_t[:], bk)
        bv_f = cpool.tile([1, C], FP, tag="bvf")
        nc.sync.dma_start(bv_f[:], bv)
        ones_row = cpool.tile([1, C], FP, tag="ones")
        nc.vector.memset(ones_row[:], 1.0)
        # bv replicated across partitions (rank-1 ones @ bv matmul)
        bv_rep = cpool.tile([128, C], FP, tag="bvrep")

        wT = {}
        with tc.tile_pool(name="wpsum", bufs=1, space="PSUM") as wpsum_pool:
            for name, wap in (("q", wq), ("k", wk), ("v", wv)):
                wt = cpool.tile([C, C], FR, tag=f"w{name}T")
                nc.sync.dma_start(wt[:], wap)
                wT[name] = wt
            pb = wpsum_pool.tile([128, C], FP, tag="wps")
            nc.tensor.matmul(pb[:], ones_row[:], bv_f[:], start=True, stop=True)
            nc.vector.tensor_copy(bv_rep[:], pb[:])
        # WvT padded to 256 columns of zeros so the float32r vT matmuls hit
        # the >=256 free-dim full-rate path (junk half never read).
        wvpad = cpool.tile([C, 2 * C], FR, tag="wvpad")
        nc.vector.memset(wvpad[:].bitcast(FP), 0.0)
        nc.vector.tensor_copy(wvpad[:, 0:C], wT["v"][:])

        # --- resident activations -----------------------------------------
        # q_sb[n]: (c, l) for this core's l-slice;  k_sb[n]: (c, m) full L;
        # vT_sb[n]: (m % 128, 32*128) i.e. 32 chunks of (m,c), all bf16.
        q_sb = [
            resid.tile([C, LSH], BF, tag=f"q{n}", name=f"q_sb{n}") for n in range(N)
        ]
        k_sb = [
            resid.tile([C, L], BF, tag=f"k{n}", name=f"k_sb{n}") for n in range(N)
        ]
        vT_sb = [
            resid.tile([128, NMT * C], BF, tag=f"v{n}", name=f"vT_sb{n}")
            for n in range(N)
        ]

        # --- fused projection + attention stream ---------------------------
        # Projections run chunk-outer (all batches per m-chunk) so attention
        # jobs for m-tile range [8b, 8b+8) unblock as soon as chunk b lands;
        # the emission interleaves them so PE never drains between phases.
        # One rotating 2-slot PSUM pool serves q/k/v projections AND scores
        # (4 banks), the AV accumulators take the other 4 banks.
        inv_sqrt_c = 1.0 / math.sqrt(C)
        wqT_r = wT["q"][:]
        wkT_r = wT["k"][:]
        wvpad_r = wvpad[:]
        with (
            tc.tile_pool(name="xin", bufs=3) as xin_pool,
            tc.tile_pool(name="wrk", bufs=3, space="PSUM") as wrk_psum,
            tc.tile_pool(name="avp", bufs=1, space="PSUM") as av_psum,
            tc.tile_pool(name="soft", bufs=1) as soft_pool,
            tc.tile_pool(name="ost", bufs=1) as ost_pool,
        ):
            def emit_q_proj(n):
                xt = xin_pool.tile([C, LSH], FR, tag="xq", bufs=2)
                nc.sync.dma_start(xt[:], xq[n])
                pq = wrk_psum.tile([128, 1024], FP, tag="ps")
                nc.tensor.matmul(
                    pq[:, 0:512], wqT_r, xt[:], start=True, stop=True
                )
                nc.scalar.activation(
                    q_sb[n][:], pq[:, 0:512], AF.Identity, bias=bq_t[:]
                )

            def emit_kv_chunk(n, bch, xt, k_act):
                # One 1024-wide m-chunk of k-hat and v-hat for batch n.
                xr_ = xt[:]
                # Both 512-wide k matmuls land in one 2-bank PSUM tile so
                # the bias-adding eviction is one 1024-wide op. That
                # eviction runs on ACT (Identity + per-partition bias),
                # which is otherwise idle while projections flow.
                pk = wrk_psum.tile([128, 1024], FP, tag="ps")
                for half in range(2):
                    nc.tensor.matmul(
                        pk[:, half * 512 : (half + 1) * 512],
                        wkT_r,
                        xr_[:, half * 512 : (half + 1) * 512],
                        start=True,
                        stop=True,
                    )
                if k_act:
                    nc.scalar.activation(
                        k_sb[n][:, bch * 1024 : (bch + 1) * 1024],
                        pk[:],
                        AF.Identity,
                        bias=bk_t[:],
                    )
                else:
                    nc.vector.tensor_scalar_add(
                        k_sb[n][:, bch * 1024 : (bch + 1) * 1024],
                        pk[:],
                        bk_t[:],
                    )
                for half in range(2):
                    ch = 2 * bch + half
                    # vT chunks: out[m,c] = sum_c' x[c',m] WvT[c',c] + bv[c].
                    # Each 128-m sub-tile occupies a 256-wide PSUM slice
                    # (f32r full-rate needs >=256 free; upper half junk).
                    pv = wrk_psum.tile([128, 1024], FP, tag="ps")
                    for sub in range(4):
                        sl = slice(sub * 256, sub * 256 + 256)
                        nc.tensor.matmul(
                            pv[:, sl],
                            xr_[:, half * 512 + sub * 128 :
                                half * 512 + (sub + 1) * 128],
                            wvpad_r,
                            start=(sub % 2 == 0),
                            stop=(sub % 2 == 1),
                        )
                    # Strided 3D read picks the real 128 of each 256; bv
                    # rides along via the broadcast add.
                    nc.vector.scalar_tensor_tensor(
                        vT_sb[n][:, ch * 512 : (ch + 1) * 512].rearrange(
                            "p (s c) -> p s c", s=4
                        ),
                        pv[:].rearrange("p (s c2) -> p s c2", s=4)[:, :, 0:128],
                        1.0,
                        bv_rep[:].unsqueeze(1).broadcast_to((128, 4, C)),
                        ALU.mult,
                        ALU.add,
                    )

            def emit_epilogue(avp_prev, lh_prev):
                # Evict the PSUM accumulator (freeing the av slot for the
                # next l-quarter) to SBUF staging on gpsimd, then DMA out.
                ob = ost_pool.tile([128, 1024], BF, tag="ob", bufs=2)
                nc.scalar.activation(ob[:], avp_prev[:], AF.Copy)
                for n in range(N):
                    nc.sync.dma_start(
                        out[n, lh_prev], ob[:, n * LHW : (n + 1) * LHW]
                    )

            pend = {}   # (lh, even mt) -> E pair tile (128, 2 x 8n x 128l)
            avps = {}   # lh -> accumulator tile

            def emit_scores(lh, mt):
                # E tiles are allocated per PAIR of consecutive m-tiles so
                # the downstream softmax elementwise ops run 2048 wide (the
                # per-op fixed costs amortize) while the PSUM stays at
                # 2-bank granularity for the 3-slot rotation.
                l0 = lh * LHW
                if mt % 2 == 0:
                    e_new = soft_pool.tile(
                        [128, 2048], BF, tag="E", bufs=E_BUFS, name="e_pair"
                    )
                    pend[(lh, mt)] = e_new
                e = pend[(lh, mt - mt % 2)]
                eh = e[:, (mt % 2) * 1024 : (mt % 2) * 1024 + 1024]
                ps = wrk_psum.tile([128, 1024], FP, tag="ps")
                for n in range(N):
                    nc.tensor.matmul(
                        ps[:, n * LHW : (n + 1) * LHW],
                        k_sb[n][:, mt * 128 : (mt + 1) * 128],
                        q_sb[n][:, l0 : l0 + LHW],
                        start=True,
                        stop=True,
                    )
                nc.scalar.activation(eh, ps[:], AF.Exp, scale=inv_sqrt_c)

            njob = [0]

            def emit_soft_av(lh, mt0, split_divide=False):
                # Softmax + AV for the job pair (lh, mt0), (lh, mt0+1).
                if mt0 == 0:
                    # One (c, 8n x 128l) accumulator, 2 PSUM banks; group
                    # start/stop is per 2KB bank (4 batch slices each).
                    avps[lh] = av_psum.tile(
                        [128, 1024], FP, tag="av", name=f"avp{lh}"
                    )
                avp = avps[lh]
                e = pend.pop((lh, mt0))
                e3 = e[:].rearrange("p (j h) -> p j h", j=2)
                # Batch-sum tree (TensorTensor, 2x packed bf16 on DVE; the
                # wide level rotates onto gpsimd every few pairs), then a
                # broadcast DIVIDE normalizes all 16 batch groups — no
                # separate reciprocal.
                s1 = soft_pool.tile([128, 1024], BF, tag="zt1", bufs=3)
                s13 = s1[:].rearrange("p (j h) -> p j h", j=2)
                nc.vector.tensor_tensor(
                    s13, e3[:, :, 0:512], e3[:, :, 512:1024], ALU.add
                )
                s2 = soft_pool.tile([128, 512], BF, tag="zt2", bufs=3)
                s23 = s2[:].rearrange("p (j h) -> p j h", j=2)
                nc.vector.tensor_tensor(
                    s23, s13[:, :, 0:256], s13[:, :, 256:512], ALU.add
                )
                zr = soft_pool.tile([128, 2 * LHW], BF, tag="zr", bufs=3)
                nc.vector.tensor_tensor(
                    zr[:].rearrange("p (j l) -> p j l", j=2),
                    s23[:, :, 0:LHW],
                    s23[:, :, LHW : 2 * LHW],
                    ALU.add,
                )
                r = soft_pool.tile([128, 2 * LHW], BF, tag="r", bufs=3)
                with nc.allow_low_precision(
                    "softmax denom is a sum of 16 O(1..500) exps; bf16 ok"
                ):
                    nc.vector.reciprocal(r[:], zr[:])
                a = soft_pool.tile([128, 2048], BF, tag="A", bufs=A_BUFS)
                # Normalize attn = E * (1/Z): per-job 3D broadcast
                # multiplies, batch groups split DVE / gpsimd.
                with nc.allow_low_precision(
                    "softmax normalize; bf16 attn weights are plenty"
                ):
                    for j in range(2):
                        ej = e[:, j * 1024 : (j + 1) * 1024].rearrange(
                            "p (g l) -> p g l", g=8
                        )
                        aj = a[:, j * 1024 : (j + 1) * 1024].rearrange(
                            "p (g l) -> p g l", g=8
                        )
                        rj = r[:, j * LHW : (j + 1) * LHW]
                        nc.vector.tensor_mul(
                            aj[:, :MUL_GD],
                            ej[:, :MUL_GD],
                            rj.unsqueeze(1).broadcast_to((128, MUL_GD, LHW)),
                        )
                        nc.gpsimd.tensor_mul(
                            aj[:, MUL_GD:],
                            ej[:, MUL_GD:],
                            rj.unsqueeze(1).broadcast_to(
                                (128, 8 - MUL_GD, LHW)
                            ),
                        )
                for j in range(2):
                    mt = mt0 + j
                    for n in range(N):
                        nc.tensor.matmul(
                            avp[:, n * LHW : (n + 1) * LHW],
                            vT_sb[n][:, mt * C : (mt + 1) * C],
                            a[:, j * 1024 + n * LHW : j * 1024 + (n + 1) * LHW],
                            start=(mt == 0 and n % 4 == 0),
                            stop=(mt == NMT - 1 and n % 4 == 3),
                        )
                if mt0 == NMT - 2:
                    emit_epilogue(avps.pop(lh), lh)

            # Emission stream: q projections, then k/v chunks interleaved
            # with the attention jobs they unblock. Chunk b (m in
            # [1024b, 1024b+1024)) enables jobs (lh=0, mt in [8b, 8b+8));
            # lh=1 jobs run after all projections. The SKEW-deep pend queue
            # software-pipelines scores against softmax/AV throughout.
            def proj_n(bch, n, k_act):
                xt = xin_pool.tile([C, 1024], FR, tag="x", bufs=3)
                nc.sync.dma_start(
                    xt[:], xk[n, :, bch * 1024 : (bch + 1) * 1024]
                )
                emit_kv_chunk(n, bch, xt, k_act)

            pairs = []

            def push_job(job, split_divide=False):
                emit_scores(*job)
                lh, mt = job
                if mt % 2 == 1:
                    pairs.append(((lh, mt - 1), split_divide))
                while 2 * len(pairs) > SKEW:
                    j, s = pairs.pop(0)
                    emit_soft_av(*j, split_divide=s)

            # bch 0 is the warmup (jobs need all 8 batches of a k/v tile):
            # no attention work exists yet, so its evictions spread across
            # DVE and POOL. bch 1..3 then interleave one batch-projection
            # per attention job (POOL takes the evictions, DVE half the v),
            # and the projection-free tail hands POOL part of the divides.
            for n in range(N):
                emit_q_proj(n)
            for n in range(N):
                proj_n(0, n, True)
            for bch in range(1, 4):
                for n in range(N):
                    proj_n(bch, n, True)
                    push_job((0, 8 * (bch - 1) + n))
            for mt in range(24, 32):
                push_job((0, mt), split_divide=True)
            for lh in range(1, NLH):
                for mt in range(32):
                    push_job((lh, mt), split_divide=True)
            for jb, s in pairs:
                emit_soft_av(*jb, split_divide=s)


_NC = None


def _get_nc():
    global _NC
    if _NC is None:
        _NC = build()
    return _NC


def kernel(x, Wq, bq, Wk, bk, Wv, bv):
    global LAST_RESULTS
    x = np.ascontiguousarray(np.asarray(x, dtype=np.float32))
    WqT = np.ascontiguousarray(np.asarray(Wq, dtype=np.float32).T)
    WkT = np.ascontiguousarray(np.asarray(Wk, dtype=np.float32).T)
    WvT = np.ascontiguousarray(np.asarray(Wv, dtype=np.float32).T)
    bq = np.asarray(bq, dtype=np.float32).reshape(C, 1)
    bk = np.asarray(bk, dtype=np.float32).reshape(C, 1)
    bv = np.asarray(bv, dtype=np.float32).reshape(1, C)

    xf = x.reshape(N, C, L)
    xflat = x.reshape(N, C * H * W)

    in_maps = []
    for d in range(NCORES):
        lo = d * LSH
        in_maps.append(
            {
                "xk": xf,
                "xq": np.ascontiguousarray(xf[:, :, lo : lo + LSH]),
                "wqt": WqT,
                "wkt": WkT,
                "wvt": WvT,
                "bq": bq,
                "bk": bk,
                "bv": bv,
            }
        )

    nc = _get_nc()
    res = run_bass_kernel_spmd(
        nc, in_maps, core_ids=list(range(NCORES)), trace=TRACE
    )
    LAST_RESULTS = res
    # Device returns attention output in (c, l)-major tiles; reinterleave to
    # the reference's flat (l, c) order and add the residual here.
    att = np.concatenate(
        [
            res.results[d]["out"]
            .astype(np.float32)
            .transpose(0, 1, 3, 2)
            .reshape(N, LSH * C)
            for d in range(NCORES)
        ],
        axis=1,
    )
    return (xflat + att).reshape(N, C, H, W)


# revision 51
# speedup vs baseline: 1.0186x; 1.0085x over previous
"""Trainium2 Bass kernel for nn_Attention_40570261078258.

Computes, for x:(8,128,64,64), Wq/Wk/Wv:(128,128), bq/bk/bv:(128,):
    xf = x.reshape(N, C, L);  L = 4096
    q/k/v = W @ xf + b                  -> (N, L, C) logical
    scores = q @ k^T / sqrt(C)          -> (N, L, L)
    attn = softmax(scores, axis=0)      # over the BATCH axis (torch legacy dim=0)
    out = attn @ v                      -> (N, L, C)
    return x + out.reshape(N, C, H, W)  # reinterpreting (L,C) memory as (C,H,W)

Sharding: the softmax couples all batch elements at each (l, m) pair, so
batch-parallel would need a 64MB denominator all-reduce. Instead we shard the
query dim L across the 8 cores: each core handles l in [d*512, (d+1)*512) for
ALL batch elements, making the softmax entirely local (no collectives).
Each core redundantly computes k/v for all of L (cheap vs. attention).

Engine assignment (per the TimelineSim cost model):
  PE   : projections (f32r) + scores (bf16) + attn@v (bf16)     ~152us
  ACT  : the 16.7M-element exp, and nothing else                ~134us
  DVE  : softmax tree + reciprocal + normalize (stt 4x paths),
         q-hat eviction, AV eviction                            ~110us
  POOL : k-hat / v-hat PSUM evictions (bias-add casts)          ~104us
The softmax elementwise chain uses scalar_tensor_tensor ops (4x DVE rate for
all-SBUF bf16 operands) instead of tensor_tensor (2x ceiling).

SPMD: all cores run the identical graph; the per-core slice is selected by the
host passing a per-core q-input slice (xq). The device returns the attention
output in (c,l)-major tiles; the host reinterleaves and adds the residual.
"""

import math

import numpy as np

import concourse.bacc as bacc
import concourse.bass as bass
import concourse.mybir as mybir
import concourse.tile as tile
from concourse.bass_utils import run_bass_kernel_spmd

N, C, H, W = 8, 128, 64, 64
L = H * W            # 4096 pixels
NCORES = 8
LSH = L // NCORES    # 512 query positions per core
NLH = 4              # l-quarters per core
LHW = LSH // NLH     # 128 l per quarter
NMT = L // 128       # 32 key/value tiles of 128

FP = mybir.dt.float32
FR = mybir.dt.float32r
BF = mybir.dt.bfloat16
AF = mybir.ActivationFunctionType
ALU = mybir.AluOpType

SKEW = 8           # jobs of lookahead between scores and softmax/AV
E_BUFS = SKEW // 2 + 3  # E pair-tile slots (4KB/partition each)
A_BUFS = 3         # attn pair-tile slots
MUL_GD = 4         # normalize-mul batch groups on DVE (rest on gpsimd)

# Set by test harness to capture a profile.
TRACE = False
LAST_RESULTS = None


def build():
    nc = bacc.Bacc(
        "TRN2",
        target_bir_lowering=False,
        debug=False,
        enable_asserts=True,
        num_devices=NCORES,
    )

    # x and the transposed weights are declared float32r (same bits as f32)
    # so the projection matmuls run at full PE rate without a bf16 pre-cast.
    xk = nc.dram_tensor("xk", [N, C, L], FR, kind="ExternalInput").ap()
    xq = nc.dram_tensor("xq", [N, C, LSH], FR, kind="ExternalInput").ap()
    # Weights arrive pre-transposed from the host: w*t[c, o] = W[o, c].
    wq = nc.dram_tensor("wqt", [C, C], FR, kind="ExternalInput").ap()
    wk = nc.dram_tensor("wkt", [C, C], FR, kind="ExternalInput").ap()
    wv = nc.dram_tensor("wvt", [C, C], FR, kind="ExternalInput").ap()
    bq = nc.dram_tensor("bq", [C, 1], FP, kind="ExternalInput").ap()
    bk = nc.dram_tensor("bk", [C, 1], FP, kind="ExternalInput").ap()
    bv = nc.dram_tensor("bv", [1, C], FP, kind="ExternalInput").ap()
    # Attention output in (c, l)-major layout, bf16 (the host upcasts during
    # the (l,c) reinterleave + residual add; out values are O(20) so bf16
    # rounding adds ~2e-3 relative error, well inside the 2e-2 gate).
    out = nc.dram_tensor("out", [N, NLH, C, LHW], BF, kind="ExternalOutput").ap()

    with tile.TileContext(nc) as tc:
        _emit(nc, tc, xk, xq, wq, wk, wv, bq, bk, bv, out)

    nc.compile()
    return nc


def _emit(nc, tc, xk, xq, wq, wk, wv, bq, bk, bv, out):
    from contextlib import ExitStack

    with ExitStack() as ctx:
        cpool = ctx.enter_context(tc.tile_pool(name="const", bufs=1))
        resid = ctx.enter_context(tc.tile_pool(name="resident", bufs=1))

        # --- constants -----------------------------------------------------
        bq_t = cpool.tile([C, 1], FP, tag="bq")
        nc.sync.dma_start(bq_t[:], bq)
        bk_t = cpool.tile([C, 1], FP, tag="bk")
        nc.sync.dma_start(bk_t[:], bk)
        bv_f = cpool.tile([1, C], FP, tag="bvf")
        nc.sync.dma_start(bv_f[:], bv)
        ones_row = cpool.tile([1, C], FP, tag="ones")
        nc.vector.memset(ones_row[:], 1.0)
        # bv replicated across partitions (rank-1 ones @ bv matmul)
        bv_rep = cpool.tile([128, C], FP, tag="bvrep")

        wT = {}
        with tc.tile_pool(name="wpsum", bufs=1, space="PSUM") as wpsum_pool:
            for name, wap in (("q", wq), ("k", wk), ("v", wv)):
                wt = cpool.tile([C, C], FR, tag=f"w{name}T")
                nc.sync.dma_start(wt[:], wap)
                wT[name] = wt
            pb = wpsum_pool.tile([128, C], FP, tag="wps")
            nc.tensor.matmul(pb[:], ones_row[:], bv_f[:], start=True, stop=True)
            nc.vector.tensor_copy(bv_rep[:], pb[:])
        # WvT padded to 256 columns of zeros so the float32r vT matmuls hit
        # the >=256 free-dim full-rate path (junk half never read).
        wvpad = cpool.tile([C, 2 * C], FR, tag="wvpad")
        nc.vector.memset(wvpad[:].bitcast(FP), 0.0)
        nc.vector.tensor_copy(wvpad[:, 0:C], wT["v"][:])

        # --- resident activations -----------------------------------------
        # q_sb[n]: (c, l) for this core's l-slice;  k_sb[n]: (c, m) full L;
        # vT_sb[n]: (m % 128, 32*128) i.e. 32 chunks of (m,c), all bf16.
        q_sb = [
            resid.tile([C, LSH], BF, tag=f"q{n}", name=f"q_sb{n}") for n in range(N)
        ]
        k_sb = [
            resid.tile([C, L], BF, tag=f"k{n}", name=f"k_sb{n}") for n in range(N)
        ]
        vT_sb = [
            resid.tile([128, NMT * C], BF, tag=f"v{n}", name=f"vT_sb{n}")
            for n in range(N)
        ]

        # --- fused projection + attention stream ---------------------------
        # Projections run chunk-outer (all batches per m-chunk) so attention
        # jobs for m-tile range [8b, 8b+8) unblock as soon as chunk b lands;
        # the emission interleaves them so PE never drains between phases.
        # One rotating 2-slot PSUM pool serves q/k/v projections AND scores
        # (4 banks), the AV accumulators take the other 4 banks.
        inv_sqrt_c = 1.0 / math.sqrt(C)
        wqT_r = wT["q"][:]
        wkT_r = wT["k"][:]
        wvpad_r = wvpad[:]
        with (
            tc.tile_pool(name="xin", bufs=3) as xin_pool,
            tc.tile_pool(name="wrk", bufs=3, space="PSUM") as wrk_psum,
            tc.tile_pool(name="avp", bufs=1, space="PSUM") as av_psum,
            tc.tile_pool(name="soft", bufs=1) as soft_pool,
            tc.tile_pool(name="ost", bufs=1) as ost_pool,
        ):
            def emit_q_proj(n):
                xt = xin_pool.tile([C, LSH], FR, tag="xq", bufs=2)
                nc.sync.dma_start(xt[:], xq[n])
                pq = wrk_psum.tile([128, 1024], FP, tag="ps")
                nc.tensor.matmul(
                    pq[:, 0:512], wqT_r, xt[:], start=True, stop=True
                )
                nc.scalar.activation(
                    q_sb[n][:], pq[:, 0:512], AF.Identity, bias=bq_t[:]
                )

            def emit_kv_chunk(n, bch, xt, k_act):
                # One 1024-wide m-chunk of k-hat and v-hat for batch n.
                xr_ = xt[:]
                # Both 512-wide k matmuls land in one 2-bank PSUM tile so
                # the bias-adding eviction is one 1024-wide op. That
                # eviction runs on ACT (Identity + per-partition bias),
                # which is otherwise idle while projections flow.
                pk = wrk_psum.tile([128, 1024], FP, tag="ps")
                for half in range(2):
                    nc.tensor.matmul(
                        pk[:, half * 512 : (half + 1) * 512],
                        wkT_r,
                        xr_[:, half * 512 : (half + 1) * 512],
                        start=True,
                        stop=True,
                    )
                if k_act:
                    nc.scalar.activation(
                        k_sb[n][:, bch * 1024 : (bch + 1) * 1024],
                        pk[:],
                        AF.Identity,
                        bias=bk_t[:],
                    )
                else:
                    nc.vector.tensor_scalar_add(
                        k_sb[n][:, bch * 1024 : (bch + 1) * 1024],
                        pk[:],
                        bk_t[:],
                    )
                for half in range(2):
                    ch = 2 * bch + half
                    # vT chunks: out[m,c] = sum_c' x[c',m] WvT[c',c] + bv[c].
                    # Each 128-m sub-tile occupies a 256-wide PSUM slice
                    # (f32r full-rate needs >=256 free; upper half junk).
                    pv = wrk_psum.tile([128, 1024], FP, tag="ps")
                    for sub in range(4):
                        sl = slice(sub * 256, sub * 256 + 256)
                        nc.tensor.matmul(
                            pv[:, sl],
                            xr_[:, half * 512 + sub * 128 :
                                half * 512 + (sub + 1) * 128],
                            wvpad_r,
                            start=(sub % 2 == 0),
                            stop=(sub % 2 == 1),
                        )
                    # Strided 3D read picks the real 128 of each 256; bv
                    # rides along via the broadcast add.
                    nc.vector.scalar_tensor_tensor(
                        vT_sb[n][:, ch * 512 : (ch + 1) * 512].rearrange(
                            "p (s c) -> p s c", s=4
                        ),
                        pv[:].rearrange("p (s c2) -> p s c2", s=4)[:, :, 0:128],
                        1.0,
                        bv_rep[:].unsqueeze(1).broadcast_to((128, 4, C)),
                        ALU.mult,
                        ALU.add,
                    )

            def emit_epilogue(avp_prev, lh_prev):
                # Evict the PSUM accumulator (freeing the av slot for the
                # next l-quarter) to SBUF staging on gpsimd, then DMA out.
                ob = ost_pool.tile([128, 1024], BF, tag="ob", bufs=2)
                nc.scalar.activation(ob[:], avp_prev[:], AF.Copy)
                for n in range(N):
                    nc.sync.dma_start(
                        out[n, lh_prev], ob[:, n * LHW : (n + 1) * LHW]
                    )

            pend = {}   # (lh, even mt) -> E pair tile (128, 2 x 8n x 128l)
            avps = {}   # lh -> accumulator tile

            def emit_scores(lh, mt):
                # E tiles are allocated per PAIR of consecutive m-tiles so
                # the downstream softmax elementwise ops run 2048 wide (the
                # per-op fixed costs amortize) while the PSUM stays at
                # 2-bank granularity for the 3-slot rotation.
                l0 = lh * LHW
                if mt % 2 == 0:
                    e_new = soft_pool.tile(
                        [128, 2048], BF, tag="E", bufs=E_BUFS, name="e_pair"
                    )
                    pend[(lh, mt)] = e_new
                e = pend[(lh, mt - mt % 2)]
                eh = e[:, (mt % 2) * 1024 : (mt % 2) * 1024 + 1024]
                ps = wrk_psum.tile([128, 1024], FP, tag="ps")
                for n in range(N):
                    nc.tensor.matmul(
                        ps[:, n * LHW : (n + 1) * LHW],
                        k_sb[n][:, mt * 128 : (mt + 1) * 128],
                        q_sb[n][:, l0 : l0 + LHW],
                        start=True,
                        stop=True,
                    )
                nc.scalar.activation(eh, ps[:], AF.Exp, scale=inv_sqrt_c)

            njob = [0]

            def emit_soft_av(lh, mt0, split_divide=False):
                # Softmax + AV for the job pair (lh, mt0), (lh, mt0+1).
                if mt0 == 0:
                    # One (c, 8n x 128l) accumulator, 2 PSUM banks; group
                    # start/stop is per 2KB bank (4 batch slices each).
                    avps[lh] = av_psum.tile(
                        [128, 1024], FP, tag="av", name=f"avp{lh}"
                    )
                avp = avps[lh]
                e = pend.pop((lh, mt0))
                e3 = e[:].rearrange("p (j h) -> p j h", j=2)
                # Batch-sum tree (TensorTensor, 2x packed bf16 on DVE; the
                # wide level rotates onto gpsimd every few pairs), then a
                # broadcast DIVIDE normalizes all 16 batch groups — no
                # separate reciprocal.
                s1 = soft_pool.tile([128, 1024], BF, tag="zt1", bufs=3)
                s13 = s1[:].rearrange("p (j h) -> p j h", j=2)
                nc.vector.tensor_tensor(
                    s13, e3[:, :, 0:512], e3[:, :, 512:1024], ALU.add
                )
                s2 = soft_pool.tile([128, 512], BF, tag="zt2", bufs=2)
                s23 = s2[:].rearrange("p (j h) -> p j h", j=2)
                nc.vector.tensor_tensor(
                    s23, s13[:, :, 0:256], s13[:, :, 256:512], ALU.add
                )
                zr = soft_pool.tile([128, 2 * LHW], BF, tag="zr", bufs=3)
                nc.vector.tensor_tensor(
                    zr[:].rearrange("p (j l) -> p j l", j=2),
                    s23[:, :, 0:LHW],
                    s23[:, :, LHW : 2 * LHW],
                    ALU.add,
                )
                r = soft_pool.tile([128, 2 * LHW], BF, tag="r", bufs=3)
                with nc.allow_low_precision(
                    "softmax denom is a sum of 16 O(1..500) exps; bf16 ok"
                ):
                    nc.vector.reciprocal(r[:], zr[:])
                a = soft_pool.tile([128, 2048], BF, tag="A", bufs=2)
                # Normalize attn = E * (1/Z): per-job 3D broadcast
                # multiplies, batch groups split DVE / gpsimd.
                with nc.allow_low_precision(
                    "softmax normalize; bf16 attn weights are plenty"
                ):
                    for j in range(2):
                        gd = MUL_GD
                        ej = e[:, j * 1024 : (j + 1) * 1024].rearrange(
                            "p (g l) -> p g l", g=8
                        )
                        aj = a[:, j * 1024 : (j + 1) * 1024].rearrange(
                            "p (g l) -> p g l", g=8
                        )
                        rj = r[:, j * LHW : (j + 1) * LHW]
                        nc.vector.tensor_mul(
                            aj[:, :gd],
                            ej[:, :gd],
                            rj.unsqueeze(1).broadcast_to((128, gd, LHW)),
                        )
                        nc.gpsimd.tensor_mul(
                            aj[:, gd:],
                            ej[:, gd:],
                            rj.unsqueeze(1).broadcast_to((128, 8 - gd, LHW)),
                        )
                for j in range(2):
                    mt = mt0 + j
                    for n in range(N):
                        nc.tensor.matmul(
                            avp[:, n * LHW : (n + 1) * LHW],
                            vT_sb[n][:, mt * C : (mt + 1) * C],
                            a[:, j * 1024 + n * LHW : j * 1024 + (n + 1) * LHW],
                            start=(mt == 0 and n % 4 == 0),
                            stop=(mt == NMT - 1 and n % 4 == 3),
                        )
                if mt0 == NMT - 2:
                    emit_epilogue(avps.pop(lh), lh)

            # Emission stream: q projections, then k/v chunks interleaved
            # with the attention jobs they unblock. Chunk b (m in
            # [1024b, 1024b+1024)) enables jobs (lh=0, mt in [8b, 8b+8));
            # lh=1 jobs run after all projections. The SKEW-deep pend queue
            # software-pipelines scores against softmax/AV throughout.
            def proj_n(bch, n, k_act):
                xt = xin_pool.tile([C, 1024], FR, tag="x", bufs=3)
                nc.sync.dma_start(
                    xt[:], xk[n, :, bch * 1024 : (bch + 1) * 1024]
                )
                emit_kv_chunk(n, bch, xt, k_act)

            pairs = []

            def push_job(job, split_divide=False):
                emit_scores(*job)
                lh, mt = job
                if mt % 2 == 1:
                    pairs.append(((lh, mt - 1), split_divide))
                while 2 * len(pairs) > SKEW:
                    j, s = pairs.pop(0)
                    emit_soft_av(*j, split_divide=s)

            # bch 0 is the warmup (jobs need all 8 batches of a k/v tile):
            # no attention work exists yet, so its evictions spread across
            # DVE and POOL. bch 1..3 then interleave one batch-projection
            # per attention job (POOL takes the evictions, DVE half the v),
            # and the projection-free tail hands POOL part of the divides.
            for n in range(N):
                emit_q_proj(n)
            for n in range(N):
                proj_n(0, n, True)
            for bch in range(1, 4):
                for n in range(N):
                    proj_n(bch, n, True)
                    push_job((0, 8 * (bch - 1) + n))
            for mt in range(24, 32):
                push_job((0, mt), split_divide=True)
            for lh in range(1, NLH):
                for mt in range(32):
                    push_job((lh, mt), split_divide=True)
            for jb, s in pairs:
                emit_soft_av(*jb, split_divide=s)


_NC = None


def _get_nc():
    global _NC
    if _NC is None:
        _NC = build()
    return _NC


def kernel(x, Wq, bq, Wk, bk, Wv, bv):
    global LAST_RESULTS
    x = np.ascontiguousarray(np.asarray(x, dtype=np.float32))
    WqT = np.ascontiguousarray(np.asarray(Wq, dtype=np.float32).T)
    WkT = np.ascontiguousarray(np.asarray(Wk, dtype=np.float32).T)
    WvT = np.ascontiguousarray(np.asarray(Wv, dtype=np.float32).T)
    bq = np.asarray(bq, dtype=np.float32).reshape(C, 1)
    bk = np.asarray(bk, dtype=np.float32).reshape(C, 1)
    bv = np.asarray(bv, dtype=np.float32).reshape(1, C)

    xf = x.reshape(N, C, L)
    xflat = x.reshape(N, C * H * W)

    in_maps = []
    for d in range(NCORES):
        lo = d * LSH
        in_maps.append(
            {
                "xk": xf,
                "xq": np.ascontiguousarray(xf[:, :, lo : lo + LSH]),
                "wqt": WqT,
                "wkt": WkT,
                "wvt": WvT,
                "bq": bq,
                "bk": bk,
                "bv": bv,
            }
        )

    nc = _get_nc()
    res = run_bass_kernel_spmd(
        nc, in_maps, core_ids=list(range(NCORES)), trace=TRACE
    )
    LAST_RESULTS = res
    # Device returns attention output in (c, l)-major tiles; reinterleave to
    # the reference's flat (l, c) order and add the residual here.
    att = np.concatenate(
        [
            res.results[d]["out"]
            .astype(np.float32)
            .transpose(0, 1, 3, 2)
            .reshape(N, LSH * C)
            for d in range(NCORES)
        ],
        axis=1,
    )
    return (xflat + att).reshape(N, C, H, W)
